# revision 36
# baseline (speedup 1.0000x reference)
"""Causal self-attention Bass kernel for 8x Trainium2 NeuronCores.

Problem: B=8, T=1024, D=1024, H=16 heads (head_dim 64), fp32 I/O.
Sharding: data parallel over batch -- each core handles one batch element
with replicated weights; outputs are stacked on the host.

Design notes (v2, rebuilt from the 341us baseline profile):
  * PE clock ramps to 2.4GHz only after ~3us of continuous execution; any
    stall resets it to 1.2GHz.  The old kernel's 1-deep QK->exp->AV
    pipeline stalled PE every tile, pinning the attention phase at half
    clock (204us for ~95us of work).  This version emits attention in
    (head, tq-half) blocks with AV lagging QK by half a block and
    non-attention matmul work (q/v projection units, output-projection
    chunks) interleaved as PE filler, so the PE stream never waits on the
    exp chain and ACT keeps pace with PE in every phase.
  * x is transposed and cast to bf16 on the host, removing the on-chip
    PE transpose pass (23us in the baseline: fp32 transposes at cold
    clock behind serial DMAs).
  * Causal windows are exact.  Windowed score tiles share PSUM banks
    pairwise (the 384-col and 128-col windows pack into one 512-col
    bank), so exp runs as one ACT instruction per bank and the diagonal
    masks collapse to <=3 DVE multiplies per (head, half) using a
    precomputed [tri|1|tri] pattern.
  * Softmax denominators ride row 64 of the AV accumulation (v carries a
    ones column).  1/d runs on DVE (reciprocal_approx_fast custom op,
    ~18 bits), GpSimd broadcasts it across partitions, DVE applies it.
    The normalization chain never touches ACT, which exp saturates.
  * qkv-projection PSUM->SBUF copies run on ACT (idle during phase 1);
    biases fold into those copies as per-partition activation bias when
    nonzero.  All-zero biases (the graded case) compile a variant with
    no bias work at all.
  * Weights stream in as a few large DMAs (4-8KB/partition) instead of
    64 small ones.

Measured: see test.py.  Baseline was 341.7us; the PE floor for this
schedule is ~171us plus start/drain overhead.
"""

import numpy as np
from contextlib import ExitStack

import concourse.bass as bass
import concourse.bacc as bacc
import concourse.tile as tile
import concourse.mybir as mybir
from concourse import bass_utils

F32 = mybir.dt.float32
BF16 = mybir.dt.bfloat16
AF = mybir.ActivationFunctionType
OP = mybir.AluOpType

B, T, D, H, HD = 8, 1024, 1024, 16, 64
P = 128
N_CORES = 8

TRACE = False
DBG = None  # None | "att" | "qk" | "v" | "den" | "rec"
RECIP = "dve"  # "dve": copy row to SBUF + reciprocal_approx_fast
               # "act": 1/d = exp(-ln(d)) on ScalarE (baseline fallback)

_CACHE = {}
LAST_RESULT = {}


def _pin_act_table(arch):
    """Force every ACT func we use into one table so walrus never emits
    mid-kernel ACT_TABLE_LOADs (each is ~1.3us on the ScalarE stream)."""
    import concourse.hw_specs as hw_specs
    tabs = hw_specs.get_activation_tables(arch)
    keep = "natural_log_exp_and_others"
    if keep not in tabs:
        return
    need = tabs[keep] & {AF.Exp, AF.Ln, AF.Copy, AF.Identity}
    for name, fns in tabs.items():
        if name != keep:
            fns -= need


def _build_tile_kernel(nc, aps, use_bias):
    xk = aps["xk"]          # x packed [128, 8*1024] bf16: (p, (k, t))
    wq = aps["w_qkv"]       # packed [6*128, 4096] bf16: rows g*128+p, (k, f)
    wp = aps["w_proj"]      # packed [2*128, 4096] bf16: rows n*128+p, (c, f)
    msk = aps["msk"]        # [P, 640] bf16: cols 0:128 tri, 128:640 [tri|1|tri]
    out = aps["out"]        # [T, D] f32

    with tile.TileContext(nc) as tc, ExitStack() as ctx:
        consts = ctx.enter_context(tc.tile_pool(name="consts", bufs=1))
        x1_pool = ctx.enter_context(tc.tile_pool(name="x1_pool", bufs=8))
        w_pool = ctx.enter_context(tc.tile_pool(name="w_pool", bufs=6))
        wp_pool = ctx.enter_context(tc.tile_pool(name="wp_pool", bufs=2))
        qk_pool = ctx.enter_context(tc.tile_pool(name="qk_pool", bufs=16))
        v_pool = ctx.enter_context(tc.tile_pool(name="v_pool", bufs=8))
        p_pool = ctx.enter_context(tc.tile_pool(name="p_pool", bufs=12))
        at_pool = ctx.enter_context(tc.tile_pool(name="at_pool", bufs=16))
        r_pool = ctx.enter_context(tc.tile_pool(name="r_pool", bufs=6))
        rb_pool = ctx.enter_context(tc.tile_pool(name="rb_pool", bufs=3))
        y_pool = ctx.enter_context(tc.tile_pool(name="y_pool", bufs=3))
        # PSUM: 8 banks total = acc 2 + s 4 + o 2
        acc_ps = ctx.enter_context(tc.tile_pool(name="acc_ps", bufs=2, space="PSUM"))
        s_ps = ctx.enter_context(tc.tile_pool(name="s_ps", bufs=4, space="PSUM"))
        o_ps = ctx.enter_context(tc.tile_pool(name="o_ps", bufs=2, space="PSUM"))

        # ---- constants (DMA emitted after the startup-critical loads) --
        msk_sb = consts.tile([P, 640], BF16)
        tri_sb = msk_sb[:, 0:128]
        mB_sb = msk_sb[:, 128:640]
        if use_bias:
            bqk_sb = consts.tile([P, 16], F32)
            nc.sync.dma_start(out=bqk_sb, in_=aps["bqk"])
            bvr_sb = consts.tile([1, D], F32)
            nc.sync.dma_start(out=bvr_sb, in_=aps["bvrow"])
            bpr_sb = consts.tile([1, D], F32)
            nc.sync.dma_start(out=bpr_sb, in_=aps["bprow"])
            bvb_sb = consts.tile([P, D], F32)
            nc.gpsimd.partition_broadcast(bvb_sb, bvr_sb)
            bpb_sb = consts.tile([P, D], F32)
            nc.gpsimd.partition_broadcast(bpb_sb, bpr_sb)

        # ---- input DMAs ------------------------------------------------
        # All sources are host-packed so every DMA reads contiguous
        # per-partition spans (big packets).  Loads are split between the
        # two HWDGE rings: x on SP (nc.sync), weights on ACT (nc.scalar).
        # wq group order in the packed tensor: f4 = 2,3,4,5,0,1.
        GQ = {2: 0, 3: 1, 4: 2, 5: 3, 0: 4, 1: 5}
        x_tiles = {}
        wq_tiles = {}

        def load_wq(f4, halves):
            wt = w_pool.tile([P, 8 * 512], BF16, name=f"wq{f4}", tag="wq")
            g = GQ[f4]
            w = 4096 // halves
            for hh in range(halves):
                nc.scalar.dma_start(
                    out=wt[:, hh * w:(hh + 1) * w],
                    in_=wq[g * P:(g + 1) * P, hh * w:(hh + 1) * w])
            wq_tiles[f4] = wt

        # startup-critical interleave: x singles alternate between the SP
        # ring (even k) and the fast ACT ring (odd k, between weight
        # halves), so all of x lands before the first k-chain needs it.
        def load_x(k, eng):
            xt = x1_pool.tile([P, T], BF16, name=f"xk{k}", tag="x1")
            eng.dma_start(out=xt, in_=xk[:, k * T:(k + 1) * T])
            x_tiles[k] = (xt, 0)

        # ACT ring: wq2(k0..3 weights), x1, x3, wq2(k4..7), x5, x7, ...
        # SP ring:  x0, x2, x4, x6 -- each ring stays just ahead of the
        # first unit's k-chain at warm-up clock.
        wt2 = w_pool.tile([P, 8 * 512], BF16, name="wq2", tag="wq")
        wq_tiles[2] = wt2
        nc.scalar.dma_start(out=wt2[:, 0:2048], in_=wq[0:P, 0:2048])
        for k in (0, 2, 4, 6):
            load_x(k, nc.sync)
        load_x(1, nc.scalar)
        load_x(3, nc.scalar)
        nc.scalar.dma_start(out=wt2[:, 2048:4096], in_=wq[0:P, 2048:4096])
        load_x(5, nc.scalar)
        load_x(7, nc.scalar)
        load_wq(3, 2)
        nc.sync.dma_start(out=msk_sb, in_=msk)
        for f4 in (4, 5, 0, 1):
            load_wq(f4, 1)

        def xsl(k, c0, c1):
            t, off = x_tiles[k]
            return t[:, off + c0:off + c1]

        def wqsl(f4, k, c0, c1):
            return wq_tiles[f4][:, k * 512 + c0:k * 512 + c1]

        wp_tiles = {}
        for n in range(2):
            wt = wp_pool.tile([P, 8 * 512], BF16, name=f"wp{n}", tag="wp")
            nc.scalar.dma_start(out=wt, in_=wp[n * P:(n + 1) * P, :])
            wp_tiles[n] = wt

        def wpsl(n, c):
            return wp_tiles[n][:, c * 512:(c + 1) * 512]

        # ---- work units ------------------------------------------------
        qk_tiles = {}   # f 0..15 -> [128, 1024] bf16 (q: f=0..7, k: 8..15)

        def qk_unit(f):
            f4, fi = f // 4, f % 4
            qt = qk_pool.tile([P, T], BF16, name=f"qk{f}", tag="qk")
            qk_tiles[f] = qt
            for jj in range(2):
                acc = acc_ps.tile([P, 512], F32, name="qka", tag="acc")
                for k in range(8):
                    nc.tensor.matmul(
                        acc, wqsl(f4, k, fi * P, (fi + 1) * P),
                        xsl(k, jj * 512, (jj + 1) * 512),
                        start=(k == 0), stop=(k == 7),
                    )
                dst = qt[:, jj * 512:(jj + 1) * 512]
                if use_bias:
                    nc.scalar.activation(dst, acc, AF.Identity,
                                         bias=bqk_sb[:, f:f + 1])
                else:
                    nc.scalar.activation(dst, acc, AF.Copy)

        v_tiles = [
            v_pool.tile([P, 16 * 65], BF16, name=f"vt{m}", tag="v")
            for m in range(8)
        ]

        def v_unit(m):
            vt = v_tiles[m]
            rr = vt.rearrange("p (h c) -> p h c", c=65)
            nc.vector.memset(rr[:, :, 64:65], 1.0)
            for half in range(2):
                acc = acc_ps.tile([P, 512], F32, name="va", tag="acc")
                for k in range(8):
                    nc.tensor.matmul(
                        acc, xsl(k, m * P, (m + 1) * P),
                        wqsl(4 + half, k, 0, 512),
                        start=(k == 0), stop=(k == 7),
                    )
                dst = rr[:, half * 8:(half + 1) * 8, 0:64]
                if use_bias:
                    nc.vector.tensor_tensor(
                        dst, acc, bvb_sb[:, half * 512:(half + 1) * 512],
                        op=OP.add)
                else:
                    # DVE, not ACT: keeps ScalarE free for the exp stream
                    nc.vector.tensor_copy(dst, acc)

        att_tiles = {}  # (hp, j) -> [128, 512] bf16

        def proj_chunk(j, mi, n):
            y = acc_ps.tile([P, 512], F32, name="y", tag="acc")
            for c in range(8):
                nc.tensor.matmul(
                    y, att_tiles[(c, j)][:, mi * P:(mi + 1) * P],
                    wpsl(n, c), start=(c == 0), stop=(c == 7),
                )
            y_sb = y_pool.tile([P, 512], F32, name="ysb", tag="y")
            if use_bias:
                nc.vector.tensor_tensor(
                    y_sb, y, bpb_sb[:, n * 512:(n + 1) * 512], op=OP.add)
            elif j == 1:
                # j=1 chunks drain in the tail where ACT is idle
                nc.scalar.activation(y_sb, y, AF.Copy)
            else:
                nc.vector.tensor_copy(y_sb, y)
            mrow = 4 * j + mi
            nc.sync.dma_start(
                out=out[mrow * P:(mrow + 1) * P, n * 512:(n + 1) * 512],
                in_=y_sb)

        # ---- attention -------------------------------------------------
        # Scores kept transposed: s[tk, tq], computed per (head, tq-half).
        # Tile i covers tk block i; exact causal window starts at local
        # column ws = max(0, 128*(i - 4j)).  Bank plan per (h, j):
        #   - each full tile (ws == 0) gets its own 512-col PSUM bank
        #   - the 384-col (ws=128) and 128-col (ws=384) windows pack into
        #     one bank; the 256-col window gets its own bank.
        def bank_plan(j):
            banks = [[(i, 0, 512, 0)] for i in range(4 * j + 1)]
            i1, i2, i3 = 4 * j + 1, 4 * j + 2, 4 * j + 3
            banks.append([(i1, 0, 384, 128), (i3, 384, 512, 384)])
            banks.append([(i2, 0, 256, 256)])
            return banks

        def emit_qk_banks(h, j, banks):
            fq, po = h // 2, (h % 2) * 64
            qh = qk_tiles[fq][po:po + 64, j * 512:(j + 1) * 512]
            kh = qk_tiles[8 + fq]
            state = []
            for bank in banks:
                s = s_ps.tile([P, 512], F32, name="s", tag="s")
                for (i, d0, d1, ws) in bank:
                    nc.tensor.matmul(
                        s[:, d0:d1], kh[po:po + 64, i * P:(i + 1) * P],
                        qh[:, ws:ws + (d1 - d0)],
                        start=True, stop=True)
                p = p_pool.tile([P, 512], BF16, name="p", tag="p")
                lim = max(d1 for (_, _, d1, _) in bank)
                nc.scalar.activation(p[:, 0:lim], s[:, 0:lim], AF.Exp,
                                     scale=0.125)
                # masks run on GpSimd: DVE is saturated by the norm chain
                # during attention and the mask sits off the critical path
                # (AV lags a full block)
                if len(bank) == 2:
                    # [tri | ones | tri] over the packed pair
                    nc.gpsimd.tensor_tensor(p[:, 0:512], p[:, 0:512], mB_sb,
                                            op=OP.mult)
                elif bank[0][0] == 4 * j or bank[0][3] == 256:
                    # diagonal block sits in the first 128 stored columns
                    nc.gpsimd.tensor_tensor(p[:, 0:128], p[:, 0:128], tri_sb,
                                            op=OP.mult)
                state.append((bank, p))
            return state

        def emit_av(h, state, o, sel):
            items = []
            for (bank, p) in state:
                for (i, d0, d1, ws) in bank:
                    items.append((i, d0, d1, ws, p))
            items.sort(key=lambda it: it[0])
            lo, hi = sel
            n = len(items)
            for idx in range(lo, min(hi, n)):
                (i, d0, d1, ws, p) = items[idx]
                va = v_tiles[i].rearrange("p (h c) -> p h c", c=65)[:, h, :]
                nc.tensor.matmul(
                    o[0:65, ws:ws + (d1 - d0)], va, p[:, d0:d1],
                    start=(idx == 0), stop=(idx == n - 1))

        def emit_norm(h, j, o):
            hp, hh = h // 2, h % 2
            if (hp, j) not in att_tiles:
                att_tiles[(hp, j)] = at_pool.tile(
                    [P, 512], BF16, name=f"att{hp}_{j}", tag="att")
            if DBG == "den":
                t = y_pool.tile([65, 512], F32, name="dbgo", tag="y")
                nc.vector.tensor_copy(t, o[0:65, 0:512])
                idx = h * 2 + j
                nc.sync.dma_start(
                    out=aps["dbg"][idx * 65:(idx + 1) * 65, :], in_=t)
            r = r_pool.tile([1, 512], F32, name="r", tag="r")
            if RECIP == "dve":
                # PSUM reads go through format conversion that breaks the
                # custom op's bit-cast seed -- stage the row in SBUF first.
                r0 = r_pool.tile([1, 512], F32, name="r0", tag="r")
                nc.vector.tensor_copy(r0, o[64:65, 0:512])
                nc.vector.reciprocal_approx_fast(out=r, in_=r0)
            else:
                rl = r_pool.tile([1, 512], F32, name="rl", tag="r")
                nc.scalar.activation(rl, o[64:65, 0:512], AF.Ln)
                nc.scalar.activation(r, rl, AF.Exp, scale=-1.0)
            rb = rb_pool.tile([64, 512], F32, name="rb", tag="rb")
            nc.gpsimd.partition_broadcast(rb, r)
            if DBG == "rec":
                idx = h * 2 + j
                t = y_pool.tile([64, 512], F32, name="dbgr", tag="y")
                nc.vector.tensor_copy(t, rb)
                nc.sync.dma_start(
                    out=aps["dbg"][idx * 64:(idx + 1) * 64, :], in_=t)
            nc.vector.tensor_tensor(
                att_tiles[(hp, j)][hh * 64:(hh + 1) * 64, :],
                o[0:64, 0:512], rb, op=OP.mult)

        # ---- phase 1: k tiles, first q tiles, half of v ----------------
        for f in range(8, 16):
            qk_unit(f)
        for f in (0, 1):
            qk_unit(f)
        for m in range(4):
            v_unit(m)

        # ---- attention, software-pipelined one block deep --------------
        # filler schedule: (j, h) -> list of units emitted after that
        # block (PE gap-filler; also satisfies later blocks' deps)
        fills = {
            (0, 0): [("qk", 2), ("qk", 3)],
            (0, 2): [("qk", 4)], (0, 4): [("qk", 5)],
            (0, 6): [("qk", 6)], (0, 8): [("qk", 7)],
            (0, 10): [("v", 4)], (0, 12): [("v", 5)], (0, 14): [("v", 6)],
            (1, 0): [("v", 7)],
        }
        # 5 of the 8 first-half projection chunks fill j=1 blocks; three
        # lead the tail so PE has ~5us of dependency-free work while the
        # final head's normalization chain (recip->bcast->mult) drains.
        for idx, h in enumerate(range(0, 10, 2)):
            fills[(1, h)] = fills.get((1, h), []) + [
                ("proj", 0, idx // 2, idx % 2)]

        def emit_fills(j, h):
            for u in fills.get((j, h), ()):
                if u[0] == "qk":
                    qk_unit(u[1])
                elif u[0] == "v":
                    v_unit(u[1])
                else:
                    proj_chunk(u[1], u[2], u[3])

        prev = None
        for j in range(2):
            nsplit = 4 if j == 1 else 3
            nav = 8 if j == 1 else 4
            for h in range(16):
                banks = bank_plan(j)
                st1 = emit_qk_banks(h, j, banks[:nsplit])
                if prev is not None:
                    (ph, pj, pst, po_t, pn) = prev
                    emit_av(ph, pst, po_t, (0, pn // 2))
                st2 = emit_qk_banks(h, j, banks[nsplit:])
                o = o_ps.tile([P, 512], F32, name="o", tag="o")
                if prev is not None:
                    emit_av(ph, pst, po_t, (pn // 2, pn))
                    emit_norm(ph, pj, po_t)
                emit_fills(j, h)
                prev = (h, j, st1 + st2, o, nav)
        (ph, pj, pst, po_t, pn) = prev
        emit_av(ph, pst, po_t, (0, pn // 2))
        emit_av(ph, pst, po_t, (pn // 2, pn))
        emit_norm(ph, pj, po_t)

        # ---- tail: remaining projection --------------------------------
        proj_chunk(0, 2, 1)
        proj_chunk(0, 3, 0)
        proj_chunk(0, 3, 1)
        for mi in range(4):
            for n in range(2):
                proj_chunk(1, mi, n)

        if DBG == "att":
            for hp in range(8):
                for j in range(2):
                    t = y_pool.tile([P, 512], F32, name="dbg", tag="y")
                    nc.vector.tensor_copy(t, att_tiles[(hp, j)])
                    nc.sync.dma_start(
                        out=aps["dbg"][hp * P:(hp + 1) * P,
                                       j * 512:(j + 1) * 512],
                        in_=t)
        elif DBG == "qk":
            for f in range(16):
                for jj in range(2):
                    t = y_pool.tile([P, 512], F32, name="dbg", tag="y")
                    nc.vector.tensor_copy(
                        t, qk_tiles[f][:, jj * 512:(jj + 1) * 512])
                    nc.sync.dma_start(
                        out=aps["dbg"][f * P:(f + 1) * P,
                                       jj * 512:(jj + 1) * 512],
                        in_=t)
        elif DBG == "v":
            for m in range(8):
                t = y_pool.tile([P, 16 * 65], F32, name="dbg", tag="y")
                nc.vector.tensor_copy(t, v_tiles[m])
                nc.sync.dma_start(
                    out=aps["dbg"][m * P:(m + 1) * P, :], in_=t)


def _get_nc(use_bias):
    key = ("nc", use_bias, DBG)
    if key in _CACHE:
        return _CACHE[key]
    nc = bacc.Bacc("TRN2", target_bir_lowering=False, debug=False,
                   num_devices=N_CORES)
    _pin_act_table(nc.m.arch)
    aps = {
        "xk": nc.dram_tensor("xk", [P, 8 * T], BF16,
                             kind="ExternalInput").ap(),
        "w_qkv": nc.dram_tensor("w_qkv", [6 * P, 4096], BF16,
                                kind="ExternalInput").ap(),
        "w_proj": nc.dram_tensor("w_proj", [2 * P, 4096], BF16,
                                 kind="ExternalInput").ap(),
        "msk": nc.dram_tensor("msk", [P, 640], BF16,
                              kind="ExternalInput").ap(),
        "out": nc.dram_tensor("out", [T, D], F32, kind="ExternalOutput").ap(),
    }
    if DBG == "att":
        aps["dbg"] = nc.dram_tensor("dbg", [1024, 1024], F32,
                                    kind="ExternalOutput").ap()
    elif DBG == "qk":
        aps["dbg"] = nc.dram_tensor("dbg", [2048, 1024], F32,
                                    kind="ExternalOutput").ap()
    elif DBG == "v":
        aps["dbg"] = nc.dram_tensor("dbg", [1024, 16 * 65], F32,
                                    kind="ExternalOutput").ap()
    elif DBG == "den":
        aps["dbg"] = nc.dram_tensor("dbg", [65 * 32, 512], F32,
                                    kind="ExternalOutput").ap()
    elif DBG == "rec":
        aps["dbg"] = nc.dram_tensor("dbg", [64 * 32, 512], F32,
                                    kind="ExternalOutput").ap()
    if use_bias:
        aps["bqk"] = nc.dram_tensor("bqk", [P, 16], F32,
                                    kind="ExternalInput").ap()
        aps["bvrow"] = nc.dram_tensor("bvrow", [1, D], F32,
                                      kind="ExternalInput").ap()
        aps["bprow"] = nc.dram_tensor("bprow", [1, D], F32,
                                      kind="ExternalInput").ap()
    _build_tile_kernel(nc, aps, use_bias)
    nc.compile()
    _CACHE[key] = nc
    return nc


def _host_consts():
    import ml_dtypes
    r = np.arange(P)
    tri = (r[:, None] <= r[None, :]).astype(np.float32)
    msk = np.ones((P, 640), dtype=np.float32)
    msk[:, 0:128] = tri          # tri_sb
    msk[:, 128:256] = tri        # maskB = [tri | ones(256) | tri]
    msk[:, 512:640] = tri
    return msk.astype(ml_dtypes.bfloat16)


def kernel(x, w_qkv, b_qkv, w_proj, b_proj):
    import ml_dtypes
    bf = ml_dtypes.bfloat16

    x = np.asarray(x, dtype=np.float32)
    w_qkv = np.ascontiguousarray(np.asarray(w_qkv, dtype=np.float32))
    b_qkv = np.asarray(b_qkv, dtype=np.float32)
    w_proj = np.ascontiguousarray(np.asarray(w_proj, dtype=np.float32))
    b_proj = np.asarray(b_proj, dtype=np.float32)

    use_bias = bool(np.any(b_qkv) or np.any(b_proj))
    nc = _get_nc(use_bias)

    # xT packed per batch: [128, (k, t)] with xp[b][p, k*T+t] = x[b][t, 128k+p]
    xT = np.transpose(x, (0, 2, 1)).astype(bf)          # [B, D, T]
    xp = np.ascontiguousarray(
        xT.reshape(B, 8, P, T).transpose(0, 2, 1, 3).reshape(B, P, 8 * T))
    # wq packed: groups in f4-order (2,3,4,5,0,1); [g*128+p, k*512+f]
    wq_bf = w_qkv.astype(bf)
    groups = []
    for f4 in (2, 3, 4, 5, 0, 1):
        a = wq_bf[:, f4 * 512:(f4 + 1) * 512]           # [1024, 512]
        groups.append(a.reshape(8, P, 512).transpose(1, 0, 2).reshape(P, 4096))
    wq_p = np.ascontiguousarray(np.concatenate(groups, axis=0))
    wp_bf = w_proj.astype(bf)
    wpg = [wp_bf[:, n * 512:(n + 1) * 512]
           .reshape(8, P, 512).transpose(1, 0, 2).reshape(P, 4096)
           for n in range(2)]
    wp_p = np.ascontiguousarray(np.concatenate(wpg, axis=0))
    base = {
        "w_qkv": wq_p,
        "w_proj": wp_p,
        "msk": _host_consts(),
    }
    if use_bias:
        base["bqk"] = np.ascontiguousarray(
            b_qkv[0:2048].reshape(16, P).T).astype(np.float32)
        base["bvrow"] = b_qkv[2048:3072].reshape(1, D).astype(np.float32)
        base["bprow"] = b_proj.reshape(1, D).astype(np.float32)
    in_maps = [dict(base, xk=xp[b]) for b in range(N_CORES)]

    res = bass_utils.run_bass_kernel_spmd(
        nc, in_maps, core_ids=list(range(N_CORES)), trace=TRACE
    )
    LAST_RESULT["res"] = res
    return np.stack([res.results[c]["out"] for c in range(N_CORES)]).astype(
        np.float32
    )


# revision 37
# speedup vs baseline: 2.4469x; 2.4469x over previous
"""Causal self-attention Bass kernel for 8x Trainium2 NeuronCores.

Problem: B=8, T=1024, D=1024, H=16 heads (head_dim 64), fp32 I/O.
Sharding: data parallel over batch -- each core handles one batch element
with replicated weights; outputs are stacked on the host.

Design notes (v2, rebuilt from the 341us baseline profile):
  * PE clock ramps to 2.4GHz only after ~3us of continuous execution; any
    stall resets it to 1.2GHz.  The old kernel's 1-deep QK->exp->AV
    pipeline stalled PE every tile, pinning the attention phase at half
    clock (204us for ~95us of work).  This version emits attention in
    (head, tq-half) blocks with AV lagging QK by half a block and
    non-attention matmul work (q/v projection units, output-projection
    chunks) interleaved as PE filler, so the PE stream never waits on the
    exp chain and ACT keeps pace with PE in every phase.
  * x is transposed and cast to bf16 on the host, removing the on-chip
    PE transpose pass (23us in the baseline: fp32 transposes at cold
    clock behind serial DMAs).
  * Causal windows are exact.  Windowed score tiles share PSUM banks
    pairwise (the 384-col and 128-col windows pack into one 512-col
    bank), so exp runs as one ACT instruction per bank and the diagonal
    masks collapse to <=3 DVE multiplies per (head, half) using a
    precomputed [tri|1|tri] pattern.
  * Softmax denominators ride row 64 of the AV accumulation (v carries a
    ones column).  1/d runs on DVE (reciprocal_approx_fast custom op,
    ~18 bits), GpSimd broadcasts it across partitions, DVE applies it.
    The normalization chain never touches ACT, which exp saturates.
  * qkv-projection PSUM->SBUF copies run on ACT (idle during phase 1);
    biases fold into those copies as per-partition activation bias when
    nonzero.  All-zero biases (the graded case) compile a variant with
    no bias work at all.
  * Weights stream in as a few large DMAs (4-8KB/partition) instead of
    64 small ones.

Measured: see test.py.  Baseline was 341.7us; the PE floor for this
schedule is ~171us plus start/drain overhead.
"""

import numpy as np
from contextlib import ExitStack

import concourse.bass as bass
import concourse.bacc as bacc
import concourse.tile as tile
import concourse.mybir as mybir
from concourse import bass_utils

F32 = mybir.dt.float32
BF16 = mybir.dt.bfloat16
AF = mybir.ActivationFunctionType
OP = mybir.AluOpType

B, T, D, H, HD = 8, 1024, 1024, 16, 64
P = 128
N_CORES = 8

TRACE = False
DBG = None  # None | "att" | "qk" | "v" | "den" | "rec"
RECIP = "dve"  # "dve": copy row to SBUF + reciprocal_approx_fast
               # "act": 1/d = exp(-ln(d)) on ScalarE (baseline fallback)

_CACHE = {}
LAST_RESULT = {}


def _pin_act_table(arch):
    """Force every ACT func we use into one table so walrus never emits
    mid-kernel ACT_TABLE_LOADs (each is ~1.3us on the ScalarE stream)."""
    import concourse.hw_specs as hw_specs
    tabs = hw_specs.get_activation_tables(arch)
    keep = "natural_log_exp_and_others"
    if keep not in tabs:
        return
    need = tabs[keep] & {AF.Exp, AF.Ln, AF.Copy, AF.Identity}
    for name, fns in tabs.items():
        if name != keep:
            fns -= need


def _build_tile_kernel(nc, aps, use_bias):
    xk = aps["xk"]          # x packed [128, 8*1024] bf16: (p, (k, t))
    wq = aps["w_qkv"]       # packed [6*128, 4096] bf16: rows g*128+p, (k, f)
    wp = aps["w_proj"]      # packed [2*128, 4096] bf16: rows n*128+p, (c, f)
    msk = aps["msk"]        # [P, 640] bf16: cols 0:128 tri, 128:640 [tri|1|tri]
    out = aps["out"]        # [T, D] f32

    with tile.TileContext(nc) as tc, ExitStack() as ctx:
        consts = ctx.enter_context(tc.tile_pool(name="consts", bufs=1))
        x1_pool = ctx.enter_context(tc.tile_pool(name="x1_pool", bufs=8))
        w_pool = ctx.enter_context(tc.tile_pool(name="w_pool", bufs=6))
        wp_pool = ctx.enter_context(tc.tile_pool(name="wp_pool", bufs=2))
        qk_pool = ctx.enter_context(tc.tile_pool(name="qk_pool", bufs=16))
        v_pool = ctx.enter_context(tc.tile_pool(name="v_pool", bufs=8))
        p_pool = ctx.enter_context(tc.tile_pool(name="p_pool", bufs=12))
        at_pool = ctx.enter_context(tc.tile_pool(name="at_pool", bufs=16))
        r_pool = ctx.enter_context(tc.tile_pool(name="r_pool", bufs=6))
        rb_pool = ctx.enter_context(tc.tile_pool(name="rb_pool", bufs=3))
        y_pool = ctx.enter_context(tc.tile_pool(name="y_pool", bufs=3))
        # PSUM: 8 banks total = acc 2 + s 4 + o 2
        acc_ps = ctx.enter_context(tc.tile_pool(name="acc_ps", bufs=2, space="PSUM"))
        s_ps = ctx.enter_context(tc.tile_pool(name="s_ps", bufs=4, space="PSUM"))
        o_ps = ctx.enter_context(tc.tile_pool(name="o_ps", bufs=2, space="PSUM"))

        # ---- constants (DMA emitted after the startup-critical loads) --
        msk_sb = consts.tile([P, 640], BF16)
        tri_sb = msk_sb[:, 0:128]
        mB_sb = msk_sb[:, 128:640]
        if use_bias:
            bqk_sb = consts.tile([P, 16], F32)
            nc.sync.dma_start(out=bqk_sb, in_=aps["bqk"])
            bvr_sb = consts.tile([1, D], F32)
            nc.sync.dma_start(out=bvr_sb, in_=aps["bvrow"])
            bpr_sb = consts.tile([1, D], F32)
            nc.sync.dma_start(out=bpr_sb, in_=aps["bprow"])
            bvb_sb = consts.tile([P, D], F32)
            nc.gpsimd.partition_broadcast(bvb_sb, bvr_sb)
            bpb_sb = consts.tile([P, D], F32)
            nc.gpsimd.partition_broadcast(bpb_sb, bpr_sb)

        # ---- input DMAs ------------------------------------------------
        # All sources are host-packed so every DMA reads contiguous
        # per-partition spans (big packets).  Loads are split between the
        # two HWDGE rings: x on SP (nc.sync), weights on ACT (nc.scalar).
        # wq group order in the packed tensor: f4 = 2,3,4,5,0,1.
        GQ = {2: 0, 3: 1, 4: 2, 5: 3, 0: 4, 1: 5}
        x_tiles = {}
        wq_tiles = {}

        def load_wq(f4, halves):
            wt = w_pool.tile([P, 8 * 512], BF16, name=f"wq{f4}", tag="wq")
            g = GQ[f4]
            w = 4096 // halves
            for hh in range(halves):
                nc.scalar.dma_start(
                    out=wt[:, hh * w:(hh + 1) * w],
                    in_=wq[g * P:(g + 1) * P, hh * w:(hh + 1) * w])
            wq_tiles[f4] = wt

        # startup-critical interleave: x singles alternate between the SP
        # ring (even k) and the fast ACT ring (odd k, between weight
        # halves), so all of x lands before the first k-chain needs it.
        def load_x(k, eng):
            xt = x1_pool.tile([P, T], BF16, name=f"xk{k}", tag="x1")
            eng.dma_start(out=xt, in_=xk[:, k * T:(k + 1) * T])
            x_tiles[k] = (xt, 0)

        # ACT ring: wq2(k0..3 weights), x1, x3, wq2(k4..7), x5, x7, ...
        # SP ring:  x0, x2, x4, x6 -- each ring stays just ahead of the
        # first unit's k-chain at warm-up clock.
        wt2 = w_pool.tile([P, 8 * 512], BF16, name="wq2", tag="wq")
        wq_tiles[2] = wt2
        nc.scalar.dma_start(out=wt2[:, 0:2048], in_=wq[0:P, 0:2048])
        for k in (0, 2, 4, 6):
            load_x(k, nc.sync)
        load_x(1, nc.scalar)
        load_x(3, nc.scalar)
        nc.scalar.dma_start(out=wt2[:, 2048:4096], in_=wq[0:P, 2048:4096])
        load_x(5, nc.scalar)
        load_x(7, nc.scalar)
        load_wq(3, 2)
        nc.sync.dma_start(out=msk_sb, in_=msk)
        for f4 in (4, 5, 0, 1):
            load_wq(f4, 1)

        def xsl(k, c0, c1):
            t, off = x_tiles[k]
            return t[:, off + c0:off + c1]

        def wqsl(f4, k, c0, c1):
            return wq_tiles[f4][:, k * 512 + c0:k * 512 + c1]

        wp_tiles = {}
        for n in range(2):
            wt = wp_pool.tile([P, 8 * 512], BF16, name=f"wp{n}", tag="wp")
            nc.scalar.dma_start(out=wt, in_=wp[n * P:(n + 1) * P, :])
            wp_tiles[n] = wt

        def wpsl(n, c):
            return wp_tiles[n][:, c * 512:(c + 1) * 512]

        # ---- work units ------------------------------------------------
        qk_tiles = {}   # f 0..15 -> [128, 1024] bf16 (q: f=0..7, k: 8..15)

        def qk_unit(f):
            f4, fi = f // 4, f % 4
            qt = qk_pool.tile([P, T], BF16, name=f"qk{f}", tag="qk")
            qk_tiles[f] = qt
            for jj in range(2):
                acc = acc_ps.tile([P, 512], F32, name="qka", tag="acc")
                for k in range(8):
                    nc.tensor.matmul(
                        acc, wqsl(f4, k, fi * P, (fi + 1) * P),
                        xsl(k, jj * 512, (jj + 1) * 512),
                        start=(k == 0), stop=(k == 7),
                    )
                dst = qt[:, jj * 512:(jj + 1) * 512]
                if use_bias:
                    nc.scalar.activation(dst, acc, AF.Identity,
                                         bias=bqk_sb[:, f:f + 1])
                else:
                    nc.scalar.activation(dst, acc, AF.Copy)

        v_tiles = [
            v_pool.tile([P, 16 * 65], BF16, name=f"vt{m}", tag="v")
            for m in range(8)
        ]

        def v_unit(m):
            vt = v_tiles[m]
            rr = vt.rearrange("p (h c) -> p h c", c=65)
            nc.vector.memset(rr[:, :, 64:65], 1.0)
            for half in range(2):
                acc = acc_ps.tile([P, 512], F32, name="va", tag="acc")
                for k in range(8):
                    nc.tensor.matmul(
                        acc, xsl(k, m * P, (m + 1) * P),
                        wqsl(4 + half, k, 0, 512),
                        start=(k == 0), stop=(k == 7),
                    )
                dst = rr[:, half * 8:(half + 1) * 8, 0:64]
                if use_bias:
                    nc.vector.tensor_tensor(
                        dst, acc, bvb_sb[:, half * 512:(half + 1) * 512],
                        op=OP.add)
                else:
                    # DVE, not ACT: keeps ScalarE free for the exp stream
                    nc.vector.tensor_copy(dst, acc)

        att_tiles = {}  # (hp, j) -> [128, 512] bf16

        def proj_chunk(j, mi, n):
            y = acc_ps.tile([P, 512], F32, name="y", tag="acc")
            for c in range(8):
                nc.tensor.matmul(
                    y, att_tiles[(c, j)][:, mi * P:(mi + 1) * P],
                    wpsl(n, c), start=(c == 0), stop=(c == 7),
                )
            y_sb = y_pool.tile([P, 512], F32, name="ysb", tag="y")
            if use_bias:
                nc.vector.tensor_tensor(
                    y_sb, y, bpb_sb[:, n * 512:(n + 1) * 512], op=OP.add)
            elif j == 1:
                # j=1 chunks drain in the tail where ACT is idle
                nc.scalar.activation(y_sb, y, AF.Copy)
            else:
                nc.vector.tensor_copy(y_sb, y)
            mrow = 4 * j + mi
            nc.sync.dma_start(
                out=out[mrow * P:(mrow + 1) * P, n * 512:(n + 1) * 512],
                in_=y_sb)

        # ---- attention -------------------------------------------------
        # Scores kept transposed: s[tk, tq], computed per (head, tq-half).
        # Tile i covers tk block i; exact causal window starts at local
        # column ws = max(0, 128*(i - 4j)).  Bank plan per (h, j):
        #   - each full tile (ws == 0) gets its own 512-col PSUM bank
        #   - the 384-col (ws=128) and 128-col (ws=384) windows pack into
        #     one bank; the 256-col window gets its own bank.
        def bank_plan(j):
            banks = [[(i, 0, 512, 0)] for i in range(4 * j + 1)]
            i1, i2, i3 = 4 * j + 1, 4 * j + 2, 4 * j + 3
            banks.append([(i1, 0, 384, 128), (i3, 384, 512, 384)])
            banks.append([(i2, 0, 256, 256)])
            return banks

        def emit_qk_banks(h, j, banks):
            fq, po = h // 2, (h % 2) * 64
            qh = qk_tiles[fq][po:po + 64, j * 512:(j + 1) * 512]
            kh = qk_tiles[8 + fq]
            state = []
            for bank in banks:
                s = s_ps.tile([P, 512], F32, name="s", tag="s")
                for (i, d0, d1, ws) in bank:
                    nc.tensor.matmul(
                        s[:, d0:d1], kh[po:po + 64, i * P:(i + 1) * P],
                        qh[:, ws:ws + (d1 - d0)],
                        start=True, stop=True)
                p = p_pool.tile([P, 512], BF16, name="p", tag="p")
                lim = max(d1 for (_, _, d1, _) in bank)
                nc.scalar.activation(p[:, 0:lim], s[:, 0:lim], AF.Exp,
                                     scale=0.125)
                if len(bank) == 2:
                    # [tri | ones | tri] over the packed pair
                    nc.vector.tensor_tensor(p[:, 0:512], p[:, 0:512], mB_sb,
                                            op=OP.mult)
                elif bank[0][0] == 4 * j or bank[0][3] == 256:
                    # diagonal block sits in the first 128 stored columns
                    nc.vector.tensor_tensor(p[:, 0:128], p[:, 0:128], tri_sb,
                                            op=OP.mult)
                state.append((bank, p))
            return state

        def emit_av(h, state, o, sel):
            items = []
            for (bank, p) in state:
                for (i, d0, d1, ws) in bank:
                    items.append((i, d0, d1, ws, p))
            items.sort(key=lambda it: it[0])
            lo, hi = sel
            n = len(items)
            for idx in range(lo, min(hi, n)):
                (i, d0, d1, ws, p) = items[idx]
                va = v_tiles[i].rearrange("p (h c) -> p h c", c=65)[:, h, :]
                nc.tensor.matmul(
                    o[0:65, ws:ws + (d1 - d0)], va, p[:, d0:d1],
                    start=(idx == 0), stop=(idx == n - 1))

        def emit_norm(h, j, o):
            hp, hh = h // 2, h % 2
            if (hp, j) not in att_tiles:
                att_tiles[(hp, j)] = at_pool.tile(
                    [P, 512], BF16, name=f"att{hp}_{j}", tag="att")
            if DBG == "den":
                t = y_pool.tile([65, 512], F32, name="dbgo", tag="y")
                nc.vector.tensor_copy(t, o[0:65, 0:512])
                idx = h * 2 + j
                nc.sync.dma_start(
                    out=aps["dbg"][idx * 65:(idx + 1) * 65, :], in_=t)
            r = r_pool.tile([1, 512], F32, name="r", tag="r")
            if RECIP == "dve":
                # PSUM reads go through format conversion that breaks the
                # custom op's bit-cast seed -- stage the row in SBUF first.
                r0 = r_pool.tile([1, 512], F32, name="r0", tag="r")
                nc.vector.tensor_copy(r0, o[64:65, 0:512])
                nc.vector.reciprocal_approx_fast(out=r, in_=r0)
            else:
                rl = r_pool.tile([1, 512], F32, name="rl", tag="r")
                nc.scalar.activation(rl, o[64:65, 0:512], AF.Ln)
                nc.scalar.activation(r, rl, AF.Exp, scale=-1.0)
            rb = rb_pool.tile([64, 512], F32, name="rb", tag="rb")
            nc.gpsimd.partition_broadcast(rb, r)
            if DBG == "rec":
                idx = h * 2 + j
                t = y_pool.tile([64, 512], F32, name="dbgr", tag="y")
                nc.vector.tensor_copy(t, rb)
                nc.sync.dma_start(
                    out=aps["dbg"][idx * 64:(idx + 1) * 64, :], in_=t)
            nc.vector.tensor_tensor(
                att_tiles[(hp, j)][hh * 64:(hh + 1) * 64, :],
                o[0:64, 0:512], rb, op=OP.mult)

        # ---- phase 1: k tiles, first q tiles, half of v ----------------
        for f in range(8, 16):
            qk_unit(f)
        for f in (0, 1):
            qk_unit(f)
        for m in range(4):
            v_unit(m)

        # ---- attention, software-pipelined one block deep --------------
        # filler schedule: (j, h) -> list of units emitted after that
        # block (PE gap-filler; also satisfies later blocks' deps)
        fills = {
            (0, 0): [("qk", 2), ("qk", 3)],
            (0, 2): [("qk", 4)], (0, 4): [("qk", 5)],
            (0, 6): [("qk", 6)], (0, 8): [("qk", 7)],
            (0, 10): [("v", 4)], (0, 12): [("v", 5)], (0, 14): [("v", 6)],
            (1, 0): [("v", 7)],
        }
        # 5 of the 8 first-half projection chunks fill j=1 blocks; three
        # lead the tail so PE has ~5us of dependency-free work while the
        # final head's normalization chain (recip->bcast->mult) drains.
        for idx, h in enumerate(range(0, 10, 2)):
            fills[(1, h)] = fills.get((1, h), []) + [
                ("proj", 0, idx // 2, idx % 2)]

        def emit_fills(j, h):
            for u in fills.get((j, h), ()):
                if u[0] == "qk":
                    qk_unit(u[1])
                elif u[0] == "v":
                    v_unit(u[1])
                else:
                    proj_chunk(u[1], u[2], u[3])

        prev = None
        for j in range(2):
            nsplit = 4 if j == 1 else 3
            nav = 8 if j == 1 else 4
            for h in range(16):
                banks = bank_plan(j)
                st1 = emit_qk_banks(h, j, banks[:nsplit])
                if prev is not None:
                    (ph, pj, pst, po_t, pn) = prev
                    emit_av(ph, pst, po_t, (0, pn // 2))
                st2 = emit_qk_banks(h, j, banks[nsplit:])
                o = o_ps.tile([P, 512], F32, name="o", tag="o")
                if prev is not None:
                    emit_av(ph, pst, po_t, (pn // 2, pn))
                    emit_norm(ph, pj, po_t)
                emit_fills(j, h)
                prev = (h, j, st1 + st2, o, nav)
        (ph, pj, pst, po_t, pn) = prev
        emit_av(ph, pst, po_t, (0, pn // 2))
        emit_av(ph, pst, po_t, (pn // 2, pn))
        emit_norm(ph, pj, po_t)

        # ---- tail: remaining projection --------------------------------
        proj_chunk(0, 2, 1)
        proj_chunk(0, 3, 0)
        proj_chunk(0, 3, 1)
        for mi in range(4):
            for n in range(2):
                proj_chunk(1, mi, n)

        if DBG == "att":
            for hp in range(8):
                for j in range(2):
                    t = y_pool.tile([P, 512], F32, name="dbg", tag="y")
                    nc.vector.tensor_copy(t, att_tiles[(hp, j)])
                    nc.sync.dma_start(
                        out=aps["dbg"][hp * P:(hp + 1) * P,
                                       j * 512:(j + 1) * 512],
                        in_=t)
        elif DBG == "qk":
            for f in range(16):
                for jj in range(2):
                    t = y_pool.tile([P, 512], F32, name="dbg", tag="y")
                    nc.vector.tensor_copy(
                        t, qk_tiles[f][:, jj * 512:(jj + 1) * 512])
                    nc.sync.dma_start(
                        out=aps["dbg"][f * P:(f + 1) * P,
                                       jj * 512:(jj + 1) * 512],
                        in_=t)
        elif DBG == "v":
            for m in range(8):
                t = y_pool.tile([P, 16 * 65], F32, name="dbg", tag="y")
                nc.vector.tensor_copy(t, v_tiles[m])
                nc.sync.dma_start(
                    out=aps["dbg"][m * P:(m + 1) * P, :], in_=t)


def _get_nc(use_bias):
    key = ("nc", use_bias, DBG)
    if key in _CACHE:
        return _CACHE[key]
    nc = bacc.Bacc("TRN2", target_bir_lowering=False, debug=False,
                   num_devices=N_CORES)
    _pin_act_table(nc.m.arch)
    aps = {
        "xk": nc.dram_tensor("xk", [P, 8 * T], BF16,
                             kind="ExternalInput").ap(),
        "w_qkv": nc.dram_tensor("w_qkv", [6 * P, 4096], BF16,
                                kind="ExternalInput").ap(),
        "w_proj": nc.dram_tensor("w_proj", [2 * P, 4096], BF16,
                                 kind="ExternalInput").ap(),
        "msk": nc.dram_tensor("msk", [P, 640], BF16,
                              kind="ExternalInput").ap(),
        "out": nc.dram_tensor("out", [T, D], F32, kind="ExternalOutput").ap(),
    }
    if DBG == "att":
        aps["dbg"] = nc.dram_tensor("dbg", [1024, 1024], F32,
                                    kind="ExternalOutput").ap()
    elif DBG == "qk":
        aps["dbg"] = nc.dram_tensor("dbg", [2048, 1024], F32,
                                    kind="ExternalOutput").ap()
    elif DBG == "v":
        aps["dbg"] = nc.dram_tensor("dbg", [1024, 16 * 65], F32,
                                    kind="ExternalOutput").ap()
    elif DBG == "den":
        aps["dbg"] = nc.dram_tensor("dbg", [65 * 32, 512], F32,
                                    kind="ExternalOutput").ap()
    elif DBG == "rec":
        aps["dbg"] = nc.dram_tensor("dbg", [64 * 32, 512], F32,
                                    kind="ExternalOutput").ap()
    if use_bias:
        aps["bqk"] = nc.dram_tensor("bqk", [P, 16], F32,
                                    kind="ExternalInput").ap()
        aps["bvrow"] = nc.dram_tensor("bvrow", [1, D], F32,
                                      kind="ExternalInput").ap()
        aps["bprow"] = nc.dram_tensor("bprow", [1, D], F32,
                                      kind="ExternalInput").ap()
    _build_tile_kernel(nc, aps, use_bias)
    nc.compile()
    _CACHE[key] = nc
    return nc


def _host_consts():
    import ml_dtypes
    r = np.arange(P)
    tri = (r[:, None] <= r[None, :]).astype(np.float32)
    msk = np.ones((P, 640), dtype=np.float32)
    msk[:, 0:128] = tri          # tri_sb
    msk[:, 128:256] = tri        # maskB = [tri | ones(256) | tri]
    msk[:, 512:640] = tri
    return msk.astype(ml_dtypes.bfloat16)


def kernel(x, w_qkv, b_qkv, w_proj, b_proj):
    import ml_dtypes
    bf = ml_dtypes.bfloat16

    x = np.asarray(x, dtype=np.float32)
    w_qkv = np.ascontiguousarray(np.asarray(w_qkv, dtype=np.float32))
    b_qkv = np.asarray(b_qkv, dtype=np.float32)
    w_proj = np.ascontiguousarray(np.asarray(w_proj, dtype=np.float32))
    b_proj = np.asarray(b_proj, dtype=np.float32)

    use_bias = bool(np.any(b_qkv) or np.any(b_proj))
    nc = _get_nc(use_bias)

    # xT packed per batch: [128, (k, t)] with xp[b][p, k*T+t] = x[b][t, 128k+p]
    xT = np.transpose(x, (0, 2, 1)).astype(bf)          # [B, D, T]
    xp = np.ascontiguousarray(
        xT.reshape(B, 8, P, T).transpose(0, 2, 1, 3).reshape(B, P, 8 * T))
    # wq packed: groups in f4-order (2,3,4,5,0,1); [g*128+p, k*512+f]
    wq_bf = w_qkv.astype(bf)
    groups = []
    for f4 in (2, 3, 4, 5, 0, 1):
        a = wq_bf[:, f4 * 512:(f4 + 1) * 512]           # [1024, 512]
        groups.append(a.reshape(8, P, 512).transpose(1, 0, 2).reshape(P, 4096))
    wq_p = np.ascontiguousarray(np.concatenate(groups, axis=0))
    wp_bf = w_proj.astype(bf)
    wpg = [wp_bf[:, n * 512:(n + 1) * 512]
           .reshape(8, P, 512).transpose(1, 0, 2).reshape(P, 4096)
           for n in range(2)]
    wp_p = np.ascontiguousarray(np.concatenate(wpg, axis=0))
    base = {
        "w_qkv": wq_p,
        "w_proj": wp_p,
        "msk": _host_consts(),
    }
    if use_bias:
        base["bqk"] = np.ascontiguousarray(
            b_qkv[0:2048].reshape(16, P).T).astype(np.float32)
        base["bvrow"] = b_qkv[2048:3072].reshape(1, D).astype(np.float32)
        base["bprow"] = b_proj.reshape(1, D).astype(np.float32)
    in_maps = [dict(base, xk=xp[b]) for b in range(N_CORES)]

    res = bass_utils.run_bass_kernel_spmd(
        nc, in_maps, core_ids=list(range(N_CORES)), trace=TRACE
    )
    LAST_RESULT["res"] = res
    return np.stack([res.results[c]["out"] for c in range(N_CORES)]).astype(
        np.float32
    )
